# revision 13
# baseline (speedup 1.0000x reference)
"""Causal self-attention with RoPE on 8 Trainium2 NeuronCores.

Sharding: tensor-parallel over heads (16 heads / 8 cores = 2 heads per
core). Each core computes q/k/v projections for its 2 heads over all
batches/tokens, runs causal flash-style attention locally, and applies
its 256-row slice of the output projection, producing a PARTIAL output
[B*T, C]. The host sums the 8 partials (the all-reduce of the row-wise
sharded Wp).

Device-side layout choices:
  - bf16 operands everywhere on the matmul path (x, Wq/Wk/Wv/Wp, q/k/v,
    softmax probabilities): the PE streams bf16 at the same 1 col/cycle
    as f32r, but LDWEIGHTS drops from ~211ns (fp32 2-pass, and walrus
    runs with --enable-ldw-opt=false so it serializes with the matmul)
    to ~53-107ns via automatic Fast Weight Load. RoPE/softmax math stays
    fp32 (PSUM accumulation + fp32 elementwise), only storage is bf16.
  - x is passed pre-transposed (xT [C, B*T]) so the contraction dim C is
    the SBUF partition dim for every projection matmul.
  - Projections run token-major (psum [tok, feat]) so RoPE is a pure
    per-partition elementwise job on DVE; q/k tiles are then
    PE-transposed to feature-major [d, tok] for attention.
  - Scores are computed transposed (sT [k, q]); the softmax denominator
    is an accumulated ones-matmul on PE (lhsT = ones[128,128], so every
    PSUM partition holds the column sum -- reduction and broadcast in
    one op), then DVE reciprocal + multiply.
  - exp() runs without max-subtraction: scores are ~N(0,1) after the
    1/sqrt(hd) scale, so fp32->bf16 exp is safe.
  - The PE instruction stream is software-pipelined: in attention, the
    den/PV matmuls for k-tile kt-1 are emitted after the scores matmul
    for kt (PE works while ACT/DVE run exp/mask); in the projection
    phase, the q/k transposes for tile t-1 are emitted after the
    projection matmuls for tile t (PE never waits on RoPE); the output
    projection is interleaved per q-block to keep PE dense and HAM-warm.
"""
import sys
import types

sys.path.insert(0, "/opt/trn_rl_repo")

import numpy as np

B, T, C, H, HD = 4, 2048, 2048, 16, 128
P = 128
NCORE = 8
HPC = H // NCORE            # heads per core
DLOC = HPC * HD             # local feature width (256)
NT = B * T
KT = C // P                 # 16 contraction tiles
TB = T // P                 # 16 token tiles per batch
QB = 512                    # attention q-block width
NQB = T // QB
XBLK = 512                  # xT streaming block (tokens)
SCALE = float(1.0 / np.sqrt(HD))

LAST_EXEC_NS = None
TRACE = False

_cache = {}


def _ensure_profile_shim():
    """antenv.axon_hooks is absent from the container stub; recreate it so
    run_bass_kernel_spmd(trace=True) can reach the NTFF profile hook."""
    import antenv

    if "antenv.axon_hooks" in sys.modules:
        return
    hooks = types.ModuleType("antenv.axon_hooks")
    hooks._hook = None
    hooks.set_axon_ntff_profile_hook = lambda h: setattr(hooks, "_hook", h)
    hooks.get_axon_ntff_profile_hook = lambda: hooks._hook
    sys.modules["antenv.axon_hooks"] = hooks
    antenv.axon_hooks = hooks
    try:
        from trn_agent_boot.trn_boot import _ntff_profile_via_ctypes

        hooks.set_axon_ntff_profile_hook(
            _ntff_profile_via_ctypes("/opt/axon/libaxon_pjrt.so")
        )
    except Exception:
        pass


def _split_excess_waits(nc):
    """HW instruction structs hold ONE sync wait (EventSemaphore: two), but
    Tile sometimes emits more (matmul reading two fresh tiles, the tail
    drain waiting on the whole global clock). Hoist excess waits onto
    prefix NoOps on the same engine."""
    import concourse.mybir as mybir

    uid = [0]
    for fn in nc.m.functions:
        for blk in fn.blocks:
            out = []
            for inst in blk.instructions:
                si = inst.sync_info
                waits = list(si.on_wait) if si and si.on_wait else []
                cap = 2 if inst.opcode == "EventSemaphore" else 1
                if len(waits) > cap:
                    keep = waits[-cap:]
                    for w in waits[:-cap]:
                        uid[0] += 1
                        out.append(
                            mybir.InstNoOp(
                                name=f"I-waitsplit-{uid[0]}",
                                engine=inst.engine,
                                text_hint="waitsplit",
                                sync_info=mybir.SyncInfo(on_wait=[w], on_update=[]),
                            )
                        )
                    si.on_wait = keep
                out.append(inst)
            blk.instructions = out
    return nc


def _build_nc():
    import concourse.bass as bass
    import concourse.mybir as mybir
    from concourse.masks import make_identity
    from concourse.tile import TileContext

    f32 = mybir.dt.float32
    bf16 = mybir.dt.bfloat16
    EXP = mybir.ActivationFunctionType.Exp

    nc = bass.Bass(trn_type="TRN2", target_bir_lowering=False)
    xT = nc.dram_tensor("xT", [C, NT], bf16, kind="ExternalInput")
    wqk = nc.dram_tensor("wqk", [C, 2 * DLOC], bf16, kind="ExternalInput")
    wv = nc.dram_tensor("wv", [C, DLOC], bf16, kind="ExternalInput")
    wp = nc.dram_tensor("wp", [DLOC, C], bf16, kind="ExternalInput")
    cos2 = nc.dram_tensor("cos2", [T, P], f32, kind="ExternalInput")
    sin2 = nc.dram_tensor("sin2", [T, P], f32, kind="ExternalInput")
    tri = nc.dram_tensor("tri", [P, 640], bf16, kind="ExternalInput")
    y = nc.dram_tensor("y", [NT, C], bf16, kind="ExternalOutput")

    with nc.allow_low_precision(
        reason="bf16 matmul operands; accumulation stays fp32 in PSUM"
    ), TileContext(nc) as tc:
        from contextlib import ExitStack
        stk = ExitStack()
        wpool = stk.enter_context(tc.tile_pool(name="wpool", bufs=1))
        cpool = stk.enter_context(tc.tile_pool(name="cpool", bufs=1))
        bpool = stk.enter_context(tc.tile_pool(name="bpool", bufs=1))
        xpool = stk.enter_context(tc.tile_pool(name="xpool", bufs=2))
        rotp = stk.enter_context(tc.tile_pool(name="rotp", bufs=3))
        tmpp = stk.enter_context(tc.tile_pool(name="tmpp", bufs=1))
        ptp = stk.enter_context(tc.tile_pool(name="ptp", bufs=3))
        ysbp = stk.enter_context(tc.tile_pool(name="ysbp", bufs=2))
        rdp = stk.enter_context(tc.tile_pool(name="rdp", bufs=1))
        psproj = stk.enter_context(tc.tile_pool(name="psproj", bufs=2, space="PSUM"))
        psot = stk.enter_context(tc.tile_pool(name="psot", bufs=2, space="PSUM"))
        pssc = stk.enter_context(tc.tile_pool(name="pssc", bufs=2, space="PSUM"))
        pden = stk.enter_context(tc.tile_pool(name="pden", bufs=2, space="PSUM"))
        with stk:
            # ---- constants / weights ----
            wqk_sb = wpool.tile([P, KT, 2 * DLOC], bf16, tag="wqk")
            wv_sb = wpool.tile([P, KT, DLOC], bf16, tag="wv")
            wp_sb = wpool.tile([P, HPC, C], bf16, tag="wp")
            nc.sync.dma_start(out=wqk_sb, in_=wqk.rearrange("(t p) m -> p t m", p=P))
            nc.sync.dma_start(out=wv_sb, in_=wv.rearrange("(t p) m -> p t m", p=P))
            nc.sync.dma_start(out=wp_sb, in_=wp.rearrange("(h p) c -> p h c", p=P))
            cos_sb = cpool.tile([P, TB, P], f32, tag="cos")
            sin_sb = cpool.tile([P, TB, P], f32, tag="sin")
            nc.sync.dma_start(out=cos_sb, in_=cos2.rearrange("(t p) d -> p t d", p=P))
            nc.sync.dma_start(out=sin_sb, in_=sin2.rearrange("(t p) d -> p t d", p=P))
            tri_sb = cpool.tile([P, 640], bf16, tag="tri")
            nc.sync.dma_start(out=tri_sb, in_=tri[:, :])
            ident = cpool.tile([P, P], bf16, tag="ident")
            make_identity(nc, ident)
            ones_sb = cpool.tile([P, P], bf16, tag="ones")
            nc.gpsimd.memset(ones_sb, 1.0)

            # phase-W chunks are emitted one q-block late (the DVE
            # reciprocal+normalize of the freshly finished block needs ~4us;
            # the next attention block's matmuls cover that latency). The
            # last chunk of a batch is deferred into the next batch's
            # projection phase via `pending_w`.
            pending_w = []

            def emit_W(b, qb, oT):
                for st in range(QB // P):
                    tt = qb * (QB // P) + st
                    for co in range(C // 512):
                        y_ps = pssc.tile([P, 512], f32, tag="s512",
                                         name="y_ps")
                        for h in range(HPC):
                            nc.tensor.matmul(
                                y_ps, oT[:, h, tt * P:(tt + 1) * P],
                                wp_sb[:, h, co * 512:(co + 1) * 512],
                                start=(h == 0), stop=(h == HPC - 1),
                            )
                        y_sb = ysbp.tile([P, 512], bf16, tag="ysb",
                                         name="y_sb")
                        if (st * 4 + co) % 2 == 0:
                            nc.scalar.copy(y_sb, y_ps)
                        else:
                            nc.vector.tensor_copy(y_sb, y_ps)
                        nc.sync.dma_start(
                            out=y[b * T + tt * P:b * T + (tt + 1) * P,
                                  co * 512:(co + 1) * 512],
                            in_=y_sb,
                        )

            for b in range(B):
                qT = bpool.tile([P, HPC, T], bf16, tag="qT")
                kT = bpool.tile([P, HPC, T], bf16, tag="kT")
                vsb = bpool.tile([P, TB, DLOC], bf16, tag="v")
                oT = bpool.tile([P, HPC, T], bf16, tag="oT")

                # ---- phase P: qkv projection + rope + q/k transposes ----
                # transposes are emitted one tile late so PE never stalls
                # waiting for the DVE rope chain.
                prev_rot = None

                def emit_transpose(tt, rot):
                    tps = pssc.tile([P, QB], bf16, tag="s512")
                    for g in range(4):
                        nc.tensor.transpose(
                            tps[:, g * P:(g + 1) * P],
                            rot[:, g * P:(g + 1) * P], ident,
                        )
                    tsl = slice(tt * P, (tt + 1) * P)
                    # per-head contiguous copies, balanced over DVE/ACT
                    nc.vector.tensor_copy(qT[:, 0, tsl], tps[:, 0:P])
                    nc.scalar.copy(qT[:, 1, tsl], tps[:, P:2 * P])
                    nc.vector.tensor_copy(kT[:, 0, tsl], tps[:, 2 * P:3 * P])
                    nc.scalar.copy(kT[:, 1, tsl], tps[:, 3 * P:4 * P])

                for blk in range(T // XBLK):
                    xt = xpool.tile([P, KT, XBLK], bf16, tag="xt")
                    col0 = b * T + blk * XBLK
                    nc.sync.dma_start(
                        out=xt,
                        in_=xT[:, col0:col0 + XBLK].rearrange("(t p) n -> p t n", p=P),
                    )
                    for st in range(XBLK // P):
                        tt = (blk * XBLK) // P + st
                        xts = xt[:, :, st * P:(st + 1) * P]
                        ps_qk = psproj.tile([P, 2 * DLOC], f32, tag="proj")
                        for ci in range(KT):
                            nc.tensor.matmul(
                                ps_qk, xts[:, ci, :], wqk_sb[:, ci, :],
                                start=(ci == 0), stop=(ci == KT - 1),
                            )
                        ps_v = psproj.tile([P, 2 * DLOC], f32, tag="proj")
                        for ci in range(KT):
                            nc.tensor.matmul(
                                ps_v[:, 0:DLOC], xts[:, ci, :], wv_sb[:, ci, :],
                                start=(ci == 0), stop=(ci == KT - 1),
                            )
                        if prev_rot is not None:
                            emit_transpose(*prev_rot)
                        nc.any.tensor_copy(vsb[:, tt, :], ps_v[:, 0:DLOC])
                        # rope on q (cols 0:256) and k (cols 256:512)
                        rot = rotp.tile([P, 2 * DLOC], bf16, tag="rot")
                        cs = cos_sb[:, tt, :]
                        sn = sin_sb[:, tt, :]
                        for pj in range(2):
                            off = pj * DLOC
                            pair = ps_qk[:, off:off + DLOC].rearrange(
                                "p (d two) -> p d two", two=2
                            )
                            e = pair[:, :, 0]
                            o = pair[:, :, 1]
                            t1 = tmpp.tile([P, P], f32, tag="t1")
                            t2 = tmpp.tile([P, P], f32, tag="t2")
                            t3 = tmpp.tile([P, P], f32, tag="t3")
                            t4 = tmpp.tile([P, P], f32, tag="t4")
                            nc.vector.tensor_mul(t1, e, cs)
                            nc.vector.tensor_mul(t2, o, sn)
                            nc.vector.tensor_mul(t3, e, sn)
                            nc.vector.tensor_mul(t4, o, cs)
                            halves = rot[:, off:off + DLOC].rearrange(
                                "p (h eo d) -> p h eo d", h=HPC, eo=2
                            )
                            h2 = lambda ap: ap.rearrange("p (h d) -> p h d", h=HPC)
                            nc.vector.tensor_sub(halves[:, :, 0, :], h2(t1), h2(t2))
                            nc.vector.tensor_add(halves[:, :, 1, :], h2(t3), h2(t4))
                        prev_rot = (tt, rot)
                    if blk == 0 and pending_w:
                        pending_w.pop()()
                emit_transpose(*prev_rot)

                # ---- phase A: causal attention, qb-major, software-pipelined
                # PE stream (den/PV lag two k-tiles behind scores/exp); on
                # diagonal k-tiles only the live columns (q >= k-tile start)
                # are computed. Phase W runs one q-block late. ----
                for qb in range(NQB):
                    nkt = 4 * qb + 4
                    for h in range(HPC):
                        oT_ps = psot.tile([P, QB], f32, tag="ot")
                        den_ps = pden.tile([P, QB], f32, tag="den")
                        pend = []

                        def emit_pv(kt, pT, stop):
                            off = max(kt - 4 * qb, 0) * P
                            nc.tensor.matmul(
                                den_ps[:, off:], ones_sb, pT[:, off:],
                                start=(kt == 0), stop=stop,
                            )
                            nc.tensor.matmul(
                                oT_ps[:, off:],
                                vsb[:, kt, h * HD:(h + 1) * HD], pT[:, off:],
                                start=(kt == 0), stop=stop,
                            )

                        for kt in range(nkt):
                            a = kt - 4 * qb
                            off = max(a, 0) * P
                            s_ps = pssc.tile([P, QB], f32, tag="s512")
                            nc.tensor.matmul(
                                s_ps[:, off:], kT[:, h, kt * P:(kt + 1) * P],
                                qT[:, h, qb * QB + off:(qb + 1) * QB],
                                start=True, stop=True,
                            )
                            pT = ptp.tile([P, QB], bf16, tag="pT")
                            nc.scalar.activation(out=pT[:, off:],
                                                 in_=s_ps[:, off:], func=EXP,
                                                 scale=SCALE)
                            if a >= 0:  # diagonal tile: causal mask
                                nc.vector.tensor_mul(
                                    pT[:, off:off + P], pT[:, off:off + P],
                                    tri_sb[:, 512:640],
                                )
                            pend.append((kt, pT))
                            if len(pend) > 2:
                                emit_pv(*pend.pop(0), stop=False)
                        while pend:
                            emit_pv(*pend.pop(0), stop=(not pend))
                        rden = rdp.tile([P, QB], f32, tag="rden")
                        nc.vector.reciprocal(rden, den_ps)
                        nc.vector.tensor_mul(
                            oT[:, h, qb * QB:(qb + 1) * QB], oT_ps, rden)
                        if h == 0 and qb > 0:
                            emit_W(b, qb - 1, oT)
                pending_w.append(
                    lambda b=b, oT=oT: emit_W(b, NQB - 1, oT))
            pending_w.pop()()

    return _split_excess_waits(nc)


def kernel(**inputs):
    global LAST_EXEC_NS
    _ensure_profile_shim()
    import ml_dtypes
    from concourse.bass_utils import run_bass_kernel_spmd

    BF = np.dtype(ml_dtypes.bfloat16)
    x = np.asarray(inputs["x"], dtype=np.float32)
    Wq = np.asarray(inputs["Wq"], dtype=np.float32)
    Wk = np.asarray(inputs["Wk"], dtype=np.float32)
    Wv = np.asarray(inputs["Wv"], dtype=np.float32)
    Wp = np.asarray(inputs["Wp"], dtype=np.float32)
    rope_cos = np.asarray(inputs["rope_cos"], dtype=np.float32)
    rope_sin = np.asarray(inputs["rope_sin"], dtype=np.float32)

    xT = np.ascontiguousarray(x.reshape(NT, C).T.astype(BF))
    cos2 = np.ascontiguousarray(np.concatenate([rope_cos, rope_cos], axis=1))
    sin2 = np.ascontiguousarray(np.concatenate([rope_sin, rope_sin], axis=1))
    tri = np.zeros((P, 640), dtype=np.float32)
    ii = np.arange(P)
    tri[:, 512:] = (ii[None, :] >= ii[:, None]).astype(np.float32)
    tri = tri.astype(BF)

    in_maps = []
    for c in range(NCORE):
        rows = slice(c * DLOC, (c + 1) * DLOC)
        wqk_c = np.ascontiguousarray(
            np.concatenate([Wq[rows].T, Wk[rows].T], axis=1).astype(BF)
        )
        wv_c = np.ascontiguousarray(Wv[rows].T.astype(BF))
        wp_c = np.ascontiguousarray(Wp[:, rows].T.astype(BF))
        in_maps.append({
            "xT": xT, "wqk": wqk_c, "wv": wv_c, "wp": wp_c,
            "cos2": cos2, "sin2": sin2, "tri": tri,
        })

    if "nc" not in _cache:
        _cache["nc"] = _build_nc()
    res = run_bass_kernel_spmd(
        _cache["nc"], in_maps, core_ids=list(range(NCORE)), trace=TRACE,
    )
    LAST_EXEC_NS = res.exec_time_ns

    out = res.results[0]["y"].astype(np.float32)
    for c in range(1, NCORE):
        out += res.results[c]["y"]
    return out.reshape(B, T, C)
